# revision 25
# baseline (speedup 1.0000x reference)
"""AtomPosGNN distributed Trainium2 kernel (8 NeuronCores).

Reference computation (N=8192 nodes, H=128 features, L=4 layers):
    feat = concat(atom_pos, atom_emb)            # [N, 128]
    deg = dist_adj.sum(-1); isd = rsqrt(deg)
    for l in range(4):
        h = (feat * isd[:, None]) @ Ws[l]
        h = dist_adj @ h
        feat = softplus(h * isd[:, None] + bs[l])

Strategy (row shard, P=1024 rows per core):
  - Prep: stream the local adj row-block [1024, 8192] f32 from HBM once,
    cast to bf16, DMA-xbar-transpose into a SBUF-resident adj^T block
    [128, 64kb, 8s, 128r] (128KB/partition). deg computed on the PE with a
    ones-vector matmul over the transposed tiles.
  - Per layer: local g = (feat*isd) @ W (PE, feat^T resident layout),
    AllGather g (bf16, 256KB/rank), then y^T = sum_kb g_kb^T @ adjT_kb with
    g stationary and the resident adj^T streaming (N=512), epilogue
    softplus composed from Exp + bitcast-log + 1 Newton step (no Ln table).
  - adj is read from HBM exactly once; layers run entirely from SBUF.
"""

import os
import sys

for _p in ("/opt/trn_rl_repo",):
    if _p not in sys.path and os.path.isdir(_p):
        sys.path.insert(0, _p)

import numpy as np
import ml_dtypes

import concourse.bacc as bacc
import concourse.bass as bass
import concourse.mybir as mybir
import concourse.tile as tile
from concourse.bass_utils import run_bass_kernel_spmd

R = 8          # cores
N = 8192       # nodes
P = N // R     # local rows = 1024
H = 128        # hidden
L = 4          # layers
KB = N // 128  # 64 k-tiles
S = P // 128   # 8 strips of local rows
CH = 1024      # prep staging chunk columns
NCH = N // CH  # 4 chunks

F32 = mybir.dt.float32
BF16 = mybir.dt.bfloat16

LOG_A = float(np.log(2.0) / (1 << 23))
LOG_B = float(-np.log(2.0) * (127 + 0.0450466))

CAST_DMA = os.environ.get("K_CAST_DMA", "1") == "1"  # gpsimd cast-DMA vs DVE/ACT cast
WARM_AG = os.environ.get("K_WARM", "1") == "1"
TR_SPLIT = os.environ.get("K_TR_SPLIT", "1") == "1"

LAST_RESULT = None
_NC_CACHE = {}


def build_nc():
    nc = bacc.Bacc("TRN2", target_bir_lowering=False, debug=False, num_devices=R)

    adj_ext = nc.declare_dram_parameter("adj", [P, N], F32, isOutput=False)
    featT_ext = nc.declare_dram_parameter("featT", [H, P], F32, isOutput=False)
    ws_ext = nc.declare_dram_parameter("ws", [L, H, H], BF16, isOutput=False)
    bsT_ext = nc.declare_dram_parameter("bsT", [H, L], F32, isOutput=False)
    out_ext = nc.declare_dram_parameter("out", [H, P], F32, isOutput=True)

    with tile.TileContext(nc) as tc:
        with (
            tc.tile_pool(name="big", bufs=1) as big,
            tc.tile_pool(name="stage", bufs=8) as stage_pool,
            tc.tile_pool(name="stagef", bufs=6) as stagef_pool,
            tc.tile_pool(name="sb", bufs=1) as sb,
            tc.tile_pool(name="ftl", bufs=2) as ftl_pool,
            tc.tile_pool(name="pre", bufs=1) as pre_pool,
            tc.tile_pool(name="sp", bufs=2) as sp_pool,
            tc.tile_pool(name="gsb", bufs=1) as gsb_pool,
            tc.tile_pool(name="ps", bufs=1, space="PSUM") as ps,
            tc.tile_pool(name="psg", bufs=1, space="PSUM") as psg,
            tc.tile_pool(name="dram", bufs=1, space="DRAM") as dram,
        ):
            # ---- persistent SBUF ----
            at = big.tile([128, KB, S, 128], BF16, name="at")  # adj^T resident
            ones = sb.tile([128, 1], BF16, name="ones")
            nc.vector.memset(ones[:, :], 1.0)
            w_sb = sb.tile([128, L, H], BF16, name="w_sb")
            nc.sync.dma_start(
                out=w_sb[:, :, :],
                in_=ws_ext.rearrange("l k h -> k l h"),
            )
            bsT_sb = sb.tile([H, L], F32, name="bsT_sb")
            nc.sync.dma_start(out=bsT_sb[:, :], in_=bsT_ext[:, :])
            isd_rep = sb.tile([128, P], F32, name="isd_rep")

            # ---- prep: load + cast + transpose + deg ----
            # deg accumulated in SBUF; per-(chunk,strip) PSUM groups only.
            # (matmul start=True clears has_written for the WHOLE bank, so
            # interleaved accumulation groups sharing a bank corrupt each other)
            # deg computed as a side effect of the cast (accum_out sums the
            # copied row over the free axis) -> nothing READS `at` during
            # prep, so transposes never serialize behind deg work.
            deg_nat = sb.tile([128, S], F32, name="deg_nat")  # [p, strip]
            nc.vector.memset(deg_nat[:, :], 0.0)
            KC = CH // 128  # k-tiles per chunk
            idx = 0
            for c in range(NCH):
                for s in range(S):
                    st_bf = stage_pool.tile([128, CH], BF16, name="st_bf", tag="stbf")
                    st_f = stagef_pool.tile([128, CH], F32, name="st_f", tag="stf")
                    ld_eng = nc.scalar
                    ld_eng.dma_start(
                        out=st_f[:, :],
                        in_=adj_ext[s * 128 : (s + 1) * 128, c * CH : (c + 1) * CH],
                    )
                    dacc = stage_pool.tile([128, 1], F32, name="dacc", tag="dacc")
                    if idx % 3 == 1:
                        nc.scalar.activation(
                            st_bf[:, :], st_f[:, :],
                            mybir.ActivationFunctionType.Copy,
                            accum_out=dacc[:, :],
                        )
                    else:
                        nc.vector.tensor_scalar(
                            st_bf[:, :], st_f[:, :], 0.0, 0.0,
                            mybir.AluOpType.add, mybir.AluOpType.add,
                            accum_out=dacc[:, :],
                        )
                    nc.vector.tensor_tensor(
                        deg_nat[:, s : s + 1], deg_nat[:, s : s + 1], dacc[:, :],
                        mybir.AluOpType.add,
                    )
                    # transpose into at[:, c*KC:(c+1)*KC, s, :] (sync only:
                    # concurrent transposes on two queues deadlock the xbar)
                    nc.sync.dma_start(
                        out=at[:, c * KC : (c + 1) * KC, s, :],
                        in_=st_bf[:, :],
                        transpose=True,
                    )
                    idx += 1

            # warm up the collective path: the first collective pays a
            # ~70us cold cost. Issued AFTER the transposes in program order
            # (Tile serializes any transpose against previously-issued
            # collectives), but it still EXECUTES immediately since the
            # gpsimd stream has nothing before it.
            if WARM_AG:
                warm_in = dram.tile([64], BF16, name="warm_in")
                warm_out = dram.tile([64 * R], BF16, addr_space="Shared", name="warm_out")
                nc.gpsimd.collective_compute(
                    "AllGather",
                    mybir.AluOpType.bypass,
                    replica_groups=[list(range(R))],
                    ins=[warm_in[:]],
                    outs=[warm_out[:]],
                )

            # isd = 1/sqrt(deg): broadcast deg to all partitions via DRAM
            # bounce first, then compute on all 128 lanes (cheap)
            deg_dram = dram.tile([P], F32, name="deg_dram")
            nc.sync.dma_start(
                out=bass.AP(
                    tensor=deg_dram.tensor,
                    offset=deg_dram.offset,
                    ap=[[1, 128], [128, S], [1, 1]],
                ),
                in_=deg_nat[:, :],
            )
            nc.gpsimd.dma_start(
                out=isd_rep[:, :],
                in_=bass.AP(
                    tensor=deg_dram.tensor,
                    offset=deg_dram.offset,
                    ap=[[0, 128], [1, P]],
                ),
            )
            nc.vector.reciprocal(isd_rep[:, :], isd_rep[:, :])
            nc.scalar.sqrt(isd_rep[:, :], isd_rep[:, :])

            # ---- layers ----
            ftl = ftl_pool.tile([H, P], F32, name="ftl", tag="ftl")
            nc.sync.dma_start(out=ftl[:, :], in_=featT_ext[:, :])

            for l in range(L):
                # scaled features (bf16): ftl_s = ftl * isd
                ftl_s = pre_pool.tile([H, P], BF16, name="ftl_s", tag="ftls")
                nc.vector.tensor_tensor(
                    ftl_s[:, :], ftl[:, :], isd_rep[:, :], mybir.AluOpType.mult
                )
                # local g = (feat*isd) @ W : per node-block stationary
                g_ps = psg.tile([128, S, H], F32, name="g_ps", tag="gps")
                for nb in range(S):
                    nc.tensor.matmul(
                        g_ps[:, nb, :],
                        ftl_s[:, nb * 128 : (nb + 1) * 128],
                        w_sb[:, l, :],
                        start=True,
                        stop=True,
                    )
                g_stage = pre_pool.tile([128, S, H], BF16, name="g_stage", tag="gstage")
                nc.vector.tensor_copy(g_stage[:, :, :], g_ps[:, :, :])
                g_in = dram.tile([P, H], BF16, name=f"g_in{l}")
                nc.sync.dma_start(
                    out=g_in.rearrange("(nb p) f -> p nb f", p=128),
                    in_=g_stage[:, :, :],
                )
                g_out = dram.tile([N, H], BF16, addr_space="Shared", name=f"g_out{l}")
                nc.gpsimd.collective_compute(
                    "AllGather",
                    mybir.AluOpType.bypass,
                    replica_groups=[list(range(R))],
                    ins=[g_in[:, :]],
                    outs=[g_out[:, :]],
                )
                g_sb = gsb_pool.tile([128, KB, H], BF16, name="g_sb", tag="gsb")
                g_out_r = g_out.rearrange("(kb p) f -> p kb f", p=128)
                for kq in range(8):
                    nc.sync.dma_start(
                        out=g_sb[:, kq * 8 : (kq + 1) * 8, :],
                        in_=g_out_r[:, kq * 8 : (kq + 1) * 8, :],
                    )
                # big matmul: yT[f, m] += g[k, f]^T·... accumulate over kb
                yt_ps = psg.tile([H, P], F32, name="yt_ps", tag="ytps")
                for kb in range(KB):
                    for hh in range(2):
                        nc.tensor.matmul(
                            yt_ps[:, hh * 512 : (hh + 1) * 512],
                            g_sb[:, kb, :],
                            at[:, kb, hh * 4 : (hh + 1) * 4, :],
                            start=(kb == 0),
                            stop=(kb == KB - 1),
                        )
                # epilogue in halves: x = yT*isd ; softplus(x + b_l) composed
                ftl = ftl_pool.tile([H, P], F32, name="ftl", tag="ftl")
                HW_ = P // 4
                for hh in range(4):
                    cs = slice(hh * HW_, (hh + 1) * HW_)
                    x1 = sp_pool.tile([H, HW_], F32, name="x1", tag="sp_a")
                    nc.vector.tensor_tensor(
                        x1[:, :], yt_ps[:, cs], isd_rep[:, cs], mybir.AluOpType.mult
                    )
                    z0 = sp_pool.tile([H, HW_], F32, name="z0", tag="sp_b")
                    nc.scalar.activation(
                        z0[:, :],
                        x1[:, :],
                        mybir.ActivationFunctionType.Exp,
                        bias=bsT_sb[:, l : l + 1],
                        scale=1.0,
                    )
                    z = sp_pool.tile([H, HW_], F32, name="z", tag="sp_c")
                    nc.vector.tensor_scalar_add(z[:, :], z0[:, :], 1.0)
                    y0 = sp_pool.tile([H, HW_], F32, name="y0", tag="sp_a")
                    nc.vector.tensor_copy(y0[:, :], z[:, :].bitcast(mybir.dt.int32))
                    nc.vector.tensor_scalar(
                        y0[:, :], y0[:, :], LOG_A, LOG_B,
                        mybir.AluOpType.mult, mybir.AluOpType.add,
                    )
                    w_e = sp_pool.tile([H, HW_], F32, name="w_e", tag="sp_b")
                    nc.scalar.activation(
                        w_e[:, :], y0[:, :], mybir.ActivationFunctionType.Exp,
                        scale=-1.0,
                    )
                    t1 = sp_pool.tile([H, HW_], F32, name="t1", tag="sp_c")
                    nc.vector.tensor_tensor(
                        t1[:, :], z[:, :], w_e[:, :], mybir.AluOpType.mult
                    )
                    nc.vector.tensor_scalar_add(t1[:, :], t1[:, :], -1.0)
                    nc.vector.tensor_tensor(
                        ftl[:, cs], t1[:, :], y0[:, :], mybir.AluOpType.add
                    )

            nc.sync.dma_start(out=out_ext[:, :], in_=ftl[:, :])

    nc.compile()
    return nc


def kernel(atom_pos, atom_emb, dist_adj, Ws, bs):
    global LAST_RESULT
    atom_pos = np.asarray(atom_pos, dtype=np.float32)
    atom_emb = np.asarray(atom_emb, dtype=np.float32)
    dist_adj = np.ascontiguousarray(np.asarray(dist_adj, dtype=np.float32))
    Ws = np.asarray(Ws, dtype=np.float32)
    bs = np.asarray(bs, dtype=np.float32)

    feat = np.concatenate([atom_pos, atom_emb], axis=-1)  # [N, H]
    ws_bf = Ws.astype(ml_dtypes.bfloat16)
    bsT = np.ascontiguousarray(bs.T)  # [H, L]

    if "nc" not in _NC_CACHE:
        _NC_CACHE["nc"] = build_nc()
    nc = _NC_CACHE["nc"]

    in_maps = []
    for c in range(R):
        rows = slice(c * P, (c + 1) * P)
        in_maps.append(
            {
                "adj": np.ascontiguousarray(dist_adj[rows]),
                "featT": np.ascontiguousarray(feat[rows].T),
                "ws": ws_bf,
                "bsT": bsT,
            }
        )

    trace = os.environ.get("K_TRACE", "0") == "1"
    kw = {}
    if trace:
        kw["trace_cores"] = list(range(R))
        kw["stitch_traces"] = os.environ.get("K_STITCH", "0") == "1"
    LAST_RESULT = run_bass_kernel_spmd(
        nc, in_maps, core_ids=list(range(R)), trace=trace, **kw
    )
    outs = [LAST_RESULT.results[c]["out"] for c in range(R)]  # each [H, P]
    return np.concatenate([o.T for o in outs], axis=0).astype(np.float32)


if __name__ == "__main__":
    # tiny self-run with random data (not the reference), checks shapes only
    rng = np.random.default_rng(0)
    out = kernel(
        rng.standard_normal((N, 3)).astype(np.float32),
        rng.standard_normal((N, 125)).astype(np.float32),
        rng.random((N, N), dtype=np.float32),
        (rng.standard_normal((L, H, H)) / np.sqrt(H)).astype(np.float32),
        np.zeros((L, H), np.float32),
    )
    print("out", out.shape, out.dtype, float(np.abs(out).mean()))
